# revision 34
# baseline (speedup 1.0000x reference)
"""Bass/Tile kernel for one transformer decoder layer, data-parallel over batch.

Per-core work (one batch element): LN1 -> QKV -> causal attention with
query-axis softmax -> LN2 -> FFN(gelu).

Math note: the reference softmaxes over the QUERY axis, i.e. each key-column k
is normalized over queries q >= k:
  out[q,d] = sum_k exp(s[q,k]) * (V[k,d] / Z[k]),   Z[k] = sum_{q>=k} exp(s[q,k])
We compute ST = S^T in [k, q] layout (ST = K @ Q^T / sqrt(E)) so Z is a
free-axis reduction and the AV matmul needs no transposes.

v3 design (cost-model-driven):
  - the big GEMMs (Q/K/V projections, FFN1, FFN2) run in fp8e4 DoubleRow
    perf mode with hi/lo error compensation: A@W ~ Ah@Wh + Al@Wh + Ah@Wl
    where Xh = fp8(X), Xl = fp8(X - Xh). Each DR matmul contracts 2x128
    K at 0.5 cycles/row, so a compensated GEMM costs 0.75x its bf16 cost
    with ~bf16 accuracy (validated: rel err 5.6e-3 vs 5.9e-3 bf16).
  - weights are pre-scaled by pow2 (1024 for Wq/Wk/Wv/W1, 2048 for W2) on
    the host so the fp8 lo parts stay above the subnormal floor; descales
    fold into existing evict ops (exp scale 2^-25, gelu scale 2^-10, vp
    mult 2^-10, out evict mult 2^-11) -- no extra instructions.
  - attention interior (scores, mask-matmul, exp+Z accum, AV) stays bf16:
    per the cost model a K=64 bf16 matmul already costs the same N cycles
    as any split, and e1 hi/lo eviction would swamp DVE.
  - x is DMA'd as bf16 (host cast): halves the startup DMA and doubles
    DVE rate for LN1 stats; LN evict to bf16 keeps baseline accuracy.
  - hi/lo eviction: hi via Act (Identity/Gelu straight from PSUM), lo via
    DVE tensor_tensor(psum - hi) (split with gpsimd where DVE-bound).
  - LN affine (g,b) folded into the following projection weights on host.
  - causal truncation of score matmuls at 128-col granularity; diag-block
    mask applied by an extra matmul appended to the same PSUM group.
  - Z comes free from the exp via activation accum_out.
  - weight DMAs on the gpsimd queue; x/consts/out on sync.
"""

from contextlib import ExitStack

import numpy as np

import concourse.bass as bass
import concourse.tile as tile
import concourse.bacc as bacc
from concourse import mybir

F32 = mybir.dt.float32
BF16 = mybir.dt.bfloat16
F8 = mybir.dt.float8e4
AF = mybir.ActivationFunctionType
ALU = mybir.AluOpType
DR = mybir.MatmulPerfMode.DoubleRow

P = 128
T = 1024
E = 1024
H = 16
HD = 64
FF = 4096
NT = T // P
NE = E // P
NF = FF // P
EPS = 1e-5

SW = 1024.0       # weight pre-scale for Wq/Wk/Wv/W1 (pow2)
SW2 = 2048.0      # weight pre-scale for W2
EXP_SCALE = 1.0 / (32.0 * SW * SW)   # 2^-25: scores descale * 1/sqrt(E)
GELU_SCALE = 1.0 / SW                # 2^-10
AV_SCALE = 1.0 / SW                  # 2^-10 folded into vp
OUT_SCALE = 1.0 / SW2                # 2^-11
MASK_VAL = -6000.0 / EXP_SCALE

# packed f32 const columns
_CONST_COLS = {
    "bqt": (0, NE), "bkt": (NE, NE), "b2t": (2 * NE, NE), "b1t": (3 * NE, NF),
}
_CONST_W = 3 * NE + NF
# packed bf16 const columns
_CONSTB_COLS = {"ident": (0, P), "maskd": (P, P), "bvb": (2 * P, E)}
_CONSTB_W = 2 * P + E


def build_decoder(debug=False, reps=1, bias_free=True):
    """bias_free: V-projection bias is all-zero (checked from the actual
    inputs in kernel()), letting the V evict run on Act instead of DVE."""
    nc = bacc.Bacc(None, target_bir_lowering=False, debug=debug)

    # ---------------- DRAM I/O ----------------
    x_d = nc.dram_tensor("x", (T, E), BF16, kind="ExternalInput")
    wq_d = nc.dram_tensor("wq", (NE, P, 2, E), F8, kind="ExternalInput")
    wk_d = nc.dram_tensor("wk", (NE, P, 2, E), F8, kind="ExternalInput")
    wv_d = nc.dram_tensor("wv", (NE, P, 2, E), F8, kind="ExternalInput")
    w1_d = nc.dram_tensor("w1r", (NF, P, 2, E), F8, kind="ExternalInput")
    w2_d = nc.dram_tensor("w2r", (NE, P, 2, FF), F8, kind="ExternalInput")
    c_d = nc.dram_tensor("consts", (P, _CONST_W), F32, kind="ExternalInput")
    cb_d = nc.dram_tensor("constsb", (P, _CONSTB_W), BF16, kind="ExternalInput")
    y_d = nc.dram_tensor("yT", (E, T), BF16, kind="ExternalOutput")

    with ExitStack() as es:
        tc = es.enter_context(tile.TileContext(nc))

        const = es.enter_context(tc.tile_pool(name="const", bufs=1, side="left"))
        psq = es.enter_context(tc.tile_pool(name="psq", bufs=1, space="PSUM"))

        czb = const.tile([P, _CONSTB_W], BF16)
        cz = const.tile([P, _CONST_W], F32)

        def cview(name):
            o, w = _CONST_COLS[name]
            return cz[:, o:o + w]

        def cbview(name):
            o, w = _CONSTB_COLS[name]
            return czb[:, o:o + w]

        bq_t, bk_t, b2_t, b1_t = (cview("bqt"), cview("bkt"), cview("b2t"),
                                  cview("b1t"))
        identb, maskd, bv_b = cbview("ident"), cbview("maskd"), cbview("bvb")
        eps_t = const.tile([P, 1], F32)
        nc.vector.memset(eps_t, EPS)
        # touch the needed activation tables once so loads happen during the
        # initial DMA wait; end on Sqrt (first one LN1 needs)
        warm = const.tile([P, 1], F32)
        for fn in (AF.Exp, AF.Gelu, AF.Sqrt):
            nc.scalar.activation(out=warm, in_=eps_t, func=fn)

        def sc_tile(name):
            # 2-bank tiles: V/FFN accumulators and wide score tiles (ki<4)
            return psq.tile([P, 2 * 512], F32, tag="sc2", bufs=2, name=name)

        def sc1_tile(name):
            # 1-bank tiles: QK projection halves and narrow score tiles (ki>=4)
            return psq.tile([P, 512], F32, tag="sc1", bufs=2, name=name)

        def av_tile(name, dt=F32):
            return psq.tile([P, 512], dt, tag="avtr", bufs=2, name=name)

        rep_cm = tc.For_i(0, reps, 1) if reps > 1 else None
        if rep_cm is not None:
            rep_cm.__enter__()

        # =========== Phase 1: x DMA + LN1 + transpose + V proj, per tile =====
        # xnT holds hi/lo fp8 of xn^T: [P, hl, j(E-block), token]
        xnT_pool = tc.alloc_tile_pool(name="xnT", bufs=1, side="right")
        xnT = xnT_pool.tile([P, 2, NE, T], F8)
        w1pre_pool = tc.alloc_tile_pool(name="w1pre", bufs=1, side="left")
        w1pre = [w1pre_pool.tile([P, 2, NE, P], F8, tag=f"w1p{i}", name=f"w1p{i}")
                 for i in range(4)]
        concat_pool = tc.alloc_tile_pool(name="concat", bufs=1, side="left")
        concat = concat_pool.tile([P, NT * E], BF16)
        concat_v = concat.rearrange("p (a h d) -> p a h d", a=NT, h=H)
        vt_pool = tc.alloc_tile_pool(name="vtp", bufs=1, side="left")
        vt = [vt_pool.tile([P, E], BF16, tag=f"vt{i}", name=f"vt{i}")
              for i in range(NT)]

        def ln_stats(src_fn, ti, ln_pool, nchunk=2, xn_on_act=True):
            """LN (no affine) of one [P, E] token tile -> normalized xn tile.

            Stats on DVE. rstd = exp(-0.5*ln(var+eps)) -- Ln and Exp live in
            the same Act table as the attention exp, so no act-table switch
            ever happens between LN1 and LN2 (Sqrt would cost a 1283ns
            LoadActFuncSet each way). The -mu*rstd bias runs on idle gpsimd.
            """
            stats = ln_pool.tile([P, 4, 6], F32, tag="bnstats")
            cw = E // nchunk
            for sg in range(nchunk):
                nc.vector.bn_stats(out=stats[:, sg, :],
                                   in_=src_fn(ti)[:, sg * cw:(sg + 1) * cw])
            mv = ln_pool.tile([P, 2], F32, tag="bnmv")
            nc.vector.bn_aggr(out=mv, in_=stats[:, :nchunk, :])
            nc.scalar.activation(out=mv[:, 1:2], in_=mv[:, 1:2],
                                 func=AF.Sqrt, bias=eps_t)
            nc.vector.reciprocal(mv[:, 1:2], mv[:, 1:2])
            nb = ln_pool.tile([P, 1], F32, tag="negb")
            nc.vector.scalar_tensor_tensor(
                out=nb, in0=mv[:, 0:1], scalar=-1.0, in1=mv[:, 1:2],
                op0=ALU.mult, op1=ALU.mult)
            xn = ln_pool.tile([P, E], BF16, tag="xn", bufs=NT)
            if xn_on_act:
                for h0, h1 in ([(0, 512), (512, 1024)] if nchunk == 4
                               else [(0, 1024)]):
                    nc.scalar.activation(
                        out=xn[:, h0:h1], in_=src_fn(ti)[:, h0:h1],
                        func=AF.Identity, scale=mv[:, 1:2], bias=nb)
            else:
                nc.vector.tensor_scalar(
                    out=xn, in0=src_fn(ti), scalar1=mv[:, 0:1],
                    scalar2=mv[:, 1:2], op0=ALU.subtract, op1=ALU.mult)
            return xn

        def ln_transpose(xn, dstT, ti, tr_alloc=None):
            """Transpose xn [P,E] into hi/lo fp8 column-block ti of dstT.

            4 transposes batch into one [P,512] PSUM tile so the hi (Act)
            and lo (DVE/Pool alternating) evicts run 512 wide."""
            if tr_alloc is None:
                tr_alloc = lambda nm: av_tile(nm, dt=BF16)
            tis = slice(ti * P, (ti + 1) * P)
            for j0 in range(0, NE, 4):
                ptr = tr_alloc(f"tr{ti}_{j0}")
                for jj in range(4):
                    nc.tensor.transpose(
                        ptr[:, jj * P:(jj + 1) * P],
                        xn[:, (j0 + jj) * P:(j0 + jj + 1) * P], identb)
                ptr_v = ptr.rearrange("p (a b) -> p a b", a=4)
                hi = dstT[:, 0, j0:j0 + 4, tis]
                lo = dstT[:, 1, j0:j0 + 4, tis]
                nc.scalar.activation(out=hi, in_=ptr_v, func=AF.Identity)
                # gpsimd cannot read PSUM, so the residual runs on DVE
                nc.vector.tensor_tensor(out=lo, in0=ptr_v, in1=hi,
                                        op=ALU.subtract)

        def dr_gemm(ps, lhsT_of, rhs_of, nk, n0, n1):
            """Compensated fp8 DR accumulation into ps[:, n0:n1].

            lhsT_of(hl, kp) -> stationary AP [P, 2, M] for k-pair kp
            rhs_of(hl, kp) -> moving AP [P, 2, n1-n0] for k-pair kp
            nk: number of k-pairs. Products: (h,h), (h,l-rhs), (l-lhsT,h).
            """
            last = 3 * nk - 1
            i = 0
            for kp in range(nk):
                for (wl, xl) in ((0, 0), (0, 1), (1, 0)):
                    nc.tensor.matmul(
                        ps[:, n0:n1], lhsT=lhsT_of(wl, kp), rhs=rhs_of(xl, kp),
                        start=(i == 0), stop=(i == last), perf_mode=DR)
                    i += 1

        with tc.tile_pool(name="ph1", bufs=3, side="left") as ph1, \
             tc.tile_pool(name="xin", bufs=NT, side="left") as xin, \
             tc.tile_pool(name="wv", bufs=1, side="left") as wvp:
            x_tiles = []
            # tile 0 in quarters FIRST on the sync queue (HWDGE is FIFO --
            # anything ahead of x0 delays the whole LN1 chain)
            xt0 = xin.tile([P, E], BF16, tag="x")
            for qq in range(4):
                nc.sync.dma_start(xt0[:, qq * 256:(qq + 1) * 256],
                                  x_d[0:P, qq * 256:(qq + 1) * 256])
            x_tiles.append(xt0)
            nc.sync.dma_start(cz, c_d[:, :])
            nc.sync.dma_start(czb, cb_d[:, :])
            for ti in range(1, NT):
                xt = xin.tile([P, E], BF16, tag="x")
                for hh in range(2):
                    nc.sync.dma_start(xt[:, hh * 512:(hh + 1) * 512],
                                      x_d[ti * P:(ti + 1) * P,
                                          hh * 512:(hh + 1) * 512])
                x_tiles.append(xt)
            # prefetch the first 4 FFN1 weight slices early (sync queue is
            # idle after x); consumed at the start of the FFN
            for i in range(4):
                nc.sync.dma_start(
                    w1pre[i], w1_d[i].rearrange("p two (a b) -> p two a b", a=NE))
            wvt = wvp.tile([P, NE, 2, E], F8)
            for ko in range(NE):
                nc.sync.dma_start(
                    wvt[:, ko, :, :], wv_d[ko])
            for ti in range(NT):
                xn = ln_stats(lambda t: x_tiles[t][:, :], ti, ph1,
                              nchunk=4 if ti == 0 else 2)
                ln_transpose(xn, xnT, ti)
                # V projection for this token tile (needs xnT[ti block])
                tis = slice(ti * P, (ti + 1) * P)
                ps = sc_tile(f"psv{ti}")
                for nh in range(2):
                    dr_gemm(
                        ps,
                        lambda hl, kp: xnT[:, hl, 2 * kp:2 * kp + 2, tis],
                        lambda hl, kp: wvt[:, 2 * kp:2 * kp + 2, hl,
                                           nh * 512:(nh + 1) * 512],
                        NE // 2, nh * 512, (nh + 1) * 512)
                if bias_free:
                    nc.scalar.activation(out=vt[ti], in_=ps, func=AF.Identity)
                else:
                    nc.vector.tensor_tensor(out=vt[ti], in0=ps, in1=bv_b,
                                            op=ALU.add)

        # ====== Phase 2: per-pair {Q/K proj -> scores+exp(+Z) -> AV} ======
        qk_pool = tc.alloc_tile_pool(name="qk", bufs=2, side="left")
        wqk_pool = tc.alloc_tile_pool(name="wqk", bufs=6, side="left")
        e1p = tc.alloc_tile_pool(name="e1", bufs=32, side="left")
        vpp = tc.alloc_tile_pool(name="vp", bufs=16, side="left")
        zsp = tc.alloc_tile_pool(name="zs", bufs=4, side="left")

        def emit_qk_alloc(tt):
            qtt = qk_pool.tile([P, T], BF16, tag="qt", name=f"qt{tt}")
            ktt = qk_pool.tile([P, T], BF16, tag="kt", name=f"kt{tt}")
            wsl_k = wqk_pool.tile([P, 2, NE, P], F8, tag="wqk")
            nc.gpsimd.dma_start(
                wsl_k, wk_d[tt].rearrange("p two (a b) -> p two a b", a=NE))
            wsl_q = wqk_pool.tile([P, 2, NE, P], F8, tag="wqk")
            nc.gpsimd.dma_start(
                wsl_q, wq_d[tt].rearrange("p two (a b) -> p two a b", a=NE))
            return {"q": (qtt, wsl_q, bq_t), "k": (ktt, wsl_k, bk_t), "tt": tt}

        # group order: K half0, Q half0, Q half1, K half1
        _QK_GROUPS = (("k", 0), ("q", 0), ("q", 1), ("k", 1))

        def emit_qk_group(qk, gi):
            which, th = _QK_GROUPS[gi]
            dst, wsl, b_t = qk[which]
            tt = qk["tt"]
            ps = sc1_tile(f"psqk{tt}_{gi}")
            ths = slice(th * 512, (th + 1) * 512)
            dr_gemm(
                ps,
                lambda hl, kp: wsl[:, hl, 2 * kp:2 * kp + 2, :],
                lambda hl, kp: xnT[:, hl, 2 * kp:2 * kp + 2, ths],
                NE // 2, 0, 512)
            nc.vector.tensor_scalar(
                out=dst[:, ths], in0=ps,
                scalar1=b_t[:, tt:tt + 1], scalar2=None, op0=ALU.add)

        def emit_vprime(hp, zhs, ps_alloc=None):
            """1/Z and V' tiles for pair hp (exps already done)."""
            if ps_alloc is None:
                ps_alloc = av_tile
            out = {}
            for h in (2 * hp, 2 * hp + 1):
                rz = zsp.tile([P, NT], F32, tag="rz")
                nc.vector.reciprocal(rz, zhs[h])
                vps = []
                for ki in range(NT):
                    vp_t = vpp.tile([P, HD], BF16, tag="vp")
                    nc.vector.tensor_scalar(
                        out=vp_t, in0=vt[ki][:, h * HD:(h + 1) * HD],
                        scalar1=rz[:, ki:ki + 1], scalar2=AV_SCALE,
                        op0=ALU.mult, op1=ALU.mult)
                    vps.append(vp_t)
                out[h] = (vps, ps_alloc(f"psav{h}"))
            return out

        def emit_av_chunk(hp, e1s, vinfo, qi):
            for h in (2 * hp, 2 * hp + 1):
                vps, po_ps = vinfo[h]
                for ki in range(qi + 1):
                    nc.tensor.matmul(
                        po_ps[:, qi * HD:(qi + 1) * HD],
                        lhsT=e1s[(h, ki)][:, qi * P:(qi + 1) * P],
                        rhs=vps[ki],
                        start=(ki == 0), stop=(ki == qi))

        def emit_av_flush(hp, vinfo, qi=None):
            for h in (2 * hp, 2 * hp + 1):
                _, po_ps = vinfo[h]
                if qi is None:
                    nc.vector.tensor_copy(
                        out=concat_v[:, :, h, :],
                        in_=po_ps.rearrange("p (a d) -> p a d", a=NT))
                else:
                    nc.vector.tensor_copy(
                        out=concat_v[:, qi, h, :],
                        in_=po_ps[:, qi * HD:(qi + 1) * HD])

        def emit_scores_av(tt, qk, e1s, zhs, prev, qk_next):
            """Causal-truncated ST blocks; diag mask folded into the PSUM
            accumulation group as ident.T @ maskd; one wide exp per (h, ki)
            with Z accumulated for free. The previous pair's AV matmuls and
            the NEXT pair's QK projection groups are interleaved per-ki to
            fill the exp-paced PE stalls (they don't depend on this pair's
            activations)."""
            qtt, ktt = qk["q"][0], qk["k"][0]
            hp, e1s_prev, vinfo = prev if prev is not None else (None,) * 3
            for h in (2 * tt, 2 * tt + 1):
                zhs[h] = zsp.tile([P, NT], F32, tag="zh", name=f"zh{h}")
            for ki in range(NT):
                qs = ki * P
                chunks = []
                c0 = qs
                if c0 % 512:
                    nxt = (c0 // 512 + 1) * 512
                    chunks.append((c0, nxt))
                    c0 = nxt
                while c0 < T:
                    chunks.append((c0, c0 + 512))
                    c0 += 512
                for h in (2 * tt, 2 * tt + 1):
                    po = HD * (h % 2)
                    e1ki = e1p.tile([P, T], BF16, tag="e1t", name=f"e1_{h}_{ki}")
                    e1s[(h, ki)] = e1ki
                    if ki < 4:
                        sps = sc_tile(f"pss{h}_{ki}")
                        off = 0
                    else:
                        sps = sc1_tile(f"pss{h}_{ki}")
                        off = 512
                    for (a, b) in chunks:
                        has_diag = a == qs
                        nc.tensor.matmul(
                            sps[:, a - off:b - off],
                            lhsT=ktt[po:po + HD, ki * P:(ki + 1) * P],
                            rhs=qtt[po:po + HD, a:b],
                            start=True, stop=not has_diag)
                        if has_diag:
                            nc.tensor.matmul(
                                sps[:, qs - off:qs - off + P],
                                lhsT=identb, rhs=maskd,
                                start=False, stop=True, skip_group_check=True)
                    # Z comes free from the exp accumulator (187ns flat --
                    # cheaper than any DVE/Pool reduce of the same row)
                    nc.scalar.activation(
                        out=e1ki[:, qs:], in_=sps[:, qs - off:],
                        func=AF.Exp, scale=EXP_SCALE,
                        accum_out=zhs[h][:, ki:ki + 1])
                if qk_next is not None and ki % 2 == 1:
                    emit_qk_group(qk_next, ki // 2)
                if hp is not None:
                    emit_av_chunk(hp, e1s_prev, vinfo, ki)
            if hp is not None:
                emit_av_flush(hp, vinfo)

        e1s_by_hp = {}
        zhs = {}
        vinfo_prev = None
        qk_allocs = {0: emit_qk_alloc(0), 1: emit_qk_alloc(1)}
        qk_cur = qk_allocs[0]
        for gi in range(4):
            emit_qk_group(qk_cur, gi)
        for tt in range(H // 2):
            if tt >= 1:
                vinfo_prev = emit_vprime(tt - 1, zhs)
            if tt + 2 < H // 2:
                qk_allocs[tt + 2] = emit_qk_alloc(tt + 2)
            qk_next = qk_allocs.get(tt + 1)
            e1s_by_hp[tt] = {}
            prev = (tt - 1, e1s_by_hp.pop(tt - 1), vinfo_prev) if tt >= 1 else None
            emit_scores_av(tt, qk_cur, e1s_by_hp[tt], zhs, prev, qk_next)
            qk_cur = qk_next

        # ==== last pair's AV, per-qi flushed and interleaved with LN2 ====
        xnT_pool.release()
        anT_pool = tc.alloc_tile_pool(name="anT", bufs=1, side="right")
        anT = anT_pool.tile([P, 2, NE, T], F8)
        hp = H // 2 - 1
        vinfo = emit_vprime(hp, zhs, ps_alloc=sc1_tile)
        e1s_last = e1s_by_hp.pop(hp)
        with tc.tile_pool(name="ph4", bufs=3, side="left") as ph4:
            for qi in range(NT):
                emit_av_chunk(hp, e1s_last, vinfo, qi)
                emit_av_flush(hp, vinfo, qi=qi)
                xn = ln_stats(
                    lambda t: concat[:, t * E:(t + 1) * E], qi, ph4)
                ln_transpose(xn, anT, qi)

        zsp.release(); vpp.release(); e1p.release()
        wqk_pool.release(); qk_pool.release()
        vt_pool.release()
        concat_pool.release()

        # =========== Phase 4: FFN ===========
        out_pool = tc.alloc_tile_pool(name="outT", bufs=1, side="right")
        outT = [out_pool.tile([P, T], BF16, tag=f"o{j}", name=f"o{j}")
                for j in range(NE)]
        with tc.tile_pool(name="w1s", bufs=4, side="left") as w1s, \
             tc.tile_pool(name="w2s", bufs=3, side="left") as w2s, \
             tc.tile_pool(name="hid", bufs=1, side="left") as hidp, \
             tc.tile_pool(name="gtmp", bufs=4, side="left") as gtp:
            # hid: [P, kl(FF-block), hl, token] fp8 hi/lo
            hid = hidp.tile([P, NF, 2, T], F8)
            for fo in range(NF):
                if fo < 4:
                    w1t = w1pre[fo]
                else:
                    w1t = w1s.tile([P, 2, NE, P], F8, tag="w1")
                    nc.gpsimd.dma_start(
                        w1t, w1_d[fo].rearrange("p two (a b) -> p two a b",
                                                a=NE))
                ps = sc_tile(f"psf{fo}")
                for qt in range(4):
                    qts = slice(qt * 256, (qt + 1) * 256)
                    dr_gemm(
                        ps,
                        lambda hl, kp: w1t[:, hl, 2 * kp:2 * kp + 2, :],
                        lambda hl, kp: anT[:, hl, 2 * kp:2 * kp + 2, qts],
                        NE // 2, qt * 256, (qt + 1) * 256)
                    if qt % 2 == 1:
                        th = qt // 2
                        ths = slice(th * 512, (th + 1) * 512)
                        # one gelu off PSUM (frees the bank fast); hi copy on
                        # Act, lo residual split across DVE (th0) / Pool (th1)
                        gt = gtp.tile([P, 512], BF16, tag="gt")
                        nc.scalar.activation(
                            out=gt, in_=ps[:, ths],
                            func=AF.Gelu, scale=GELU_SCALE,
                            bias=b1_t[:, fo:fo + 1])
                        nc.scalar.activation(out=hid[:, fo, 0, ths], in_=gt,
                                             func=AF.Identity)
                        eng = nc.vector if th == 0 else nc.gpsimd
                        eng.tensor_tensor(
                            out=hid[:, fo, 1, ths], in0=gt,
                            in1=hid[:, fo, 0, ths], op=ALU.subtract)
            for eo in range(NE):
                w2t = w2s.tile([P, 2, NF, P], F8, tag="w2")
                nc.gpsimd.dma_start(
                    w2t, w2_d[eo].rearrange("p two (a b) -> p two a b", a=NF))
                ps = sc_tile(f"pso{eo}")
                # last block: finer evict/DMA chunks to shrink the tail drain
                nev = 4 if eo == NE - 1 else 2
                cw = T // nev
                for th in range(2):
                    dr_gemm(
                        ps,
                        lambda hl, kp: w2t[:, hl, 2 * kp:2 * kp + 2, :],
                        lambda hl, kp: hid[:, 2 * kp:2 * kp + 2, hl,
                                           th * 512:(th + 1) * 512],
                        NF // 2, th * 512, (th + 1) * 512)
                    for cc in range(th * nev // 2, (th + 1) * nev // 2):
                        cs = slice(cc * cw, (cc + 1) * cw)
                        if (eo + cc) % 2 == 0:
                            nc.vector.tensor_scalar(
                                out=outT[eo][:, cs], in0=ps[:, cs],
                                scalar1=OUT_SCALE, scalar2=b2_t[:, eo:eo + 1],
                                op0=ALU.mult, op1=ALU.add)
                        else:
                            nc.scalar.activation(
                                out=outT[eo][:, cs], in_=ps[:, cs],
                                func=AF.Identity, scale=OUT_SCALE,
                                bias=b2_t[:, eo:eo + 1])
                        nc.sync.dma_start(y_d[eo * P:(eo + 1) * P, cs],
                                          outT[eo][:, cs])
        w1pre_pool.release()
        out_pool.release()
        anT_pool.release()
        if rep_cm is not None:
            rep_cm.__exit__(None, None, None)

    nc.compile()
    return nc


def host_hilo(w, scale):
    """-> packed hi/lo fp8 along a new axis 0: [2, ...]"""
    import ml_dtypes
    f8 = ml_dtypes.float8_e4m3
    s = np.asarray(w, np.float32) * scale
    hi = s.astype(f8)
    lo = (s - hi.astype(np.float32)).astype(f8)
    return np.stack([hi, lo], axis=0)


def host_inputs(core_x, Wq, bq, Wk, bk, Wv, bv, W1, b1, W2, b2, g1, be1, g2, be2):
    """Per-core in_map: LN affines folded into the following projections
    (q = ln_raw @ (g1*Wq) + (be1 @ Wq + bq), where ln_raw = (x-mu)*rstd),
    weights pre-scaled by pow2 and split into fp8 hi/lo in the
    block-transposed layouts the kernel expects."""
    import ml_dtypes

    f = np.float32
    bf = ml_dtypes.bfloat16

    g1 = np.asarray(g1, f); be1 = np.asarray(be1, f)
    g2 = np.asarray(g2, f); be2 = np.asarray(be2, f)
    Wq = np.asarray(Wq, f); Wk = np.asarray(Wk, f); Wv = np.asarray(Wv, f)
    W1 = np.asarray(W1, f); W2 = np.asarray(W2, f)

    bq_e = (be1 @ Wq + np.asarray(bq, f)).astype(f)
    bk_e = (be1 @ Wk + np.asarray(bk, f)).astype(f)
    bv_e = (be1 @ Wv + np.asarray(bv, f)).astype(f)
    b1_e = (be2 @ W1 + np.asarray(b1, f)).astype(f)

    Wq_s = Wq * g1[:, None]
    Wk_s = Wk * g1[:, None]
    Wv_s = Wv * g1[:, None]
    W1_s = W1 * g2[:, None]

    consts = np.zeros((P, _CONST_W), f)

    def put(name, arr):
        o, w = _CONST_COLS[name]
        consts[:, o:o + w] = arr

    put("bqt", (SW * bq_e).reshape(NE, P).T)
    put("bkt", (SW * bk_e).reshape(NE, P).T)
    put("b2t", np.asarray(b2, f).reshape(NE, P).T)
    put("b1t", b1_e.reshape(NF, P).T)

    constsb = np.zeros((P, _CONSTB_W), f)

    def putb(name, arr):
        o, w = _CONSTB_COLS[name]
        constsb[:, o:o + w] = arr

    putb("ident", np.eye(P, dtype=f))
    putb("maskd", np.where(np.triu(np.ones((P, P), bool)), 0.0, MASK_VAL))
    putb("bvb", np.broadcast_to(SW * bv_e, (P, E)))

    def pack_blocked(Ws, scale):
        # [2, K, N] hi/lo -> (N/P, P, 2, K) with [tt, p(k-in-blk), hl, (ko q)]
        hl = host_hilo(Ws, scale)  # [2, K, N]
        K, N = Ws.shape
        nk, nn = K // P, N // P
        return np.ascontiguousarray(
            hl.reshape(2, nk, P, nn, P).transpose(3, 2, 0, 1, 4)
            .reshape(nn, P, 2, K))

    def pack_wv(Ws):
        # (NE, P, 2, E): [ko, p(k-in-blk), hl, n]
        hl = host_hilo(Ws, SW)  # [2, E(k), E(n)]
        return np.ascontiguousarray(
            hl.reshape(2, NE, P, E).transpose(1, 2, 0, 3))

    return {
        "x": np.asarray(core_x, f).astype(bf),
        "wq": pack_blocked(Wq_s, SW),
        "wk": pack_blocked(Wk_s, SW),
        "wv": pack_wv(Wv_s),
        "w1r": pack_blocked(W1_s, SW),
        "w2r": pack_blocked(W2, SW2),
        "consts": consts,
        "constsb": constsb.astype(bf),
    }


# ======================================================================
# Harness entry point: full-input kernel with internal batch sharding
# ======================================================================

_NC_CACHE = {}


def _get_nc(bias_free=True):
    key = ("nc", bias_free)
    if key not in _NC_CACHE:
        _NC_CACHE[key] = build_decoder(bias_free=bias_free)
    return _NC_CACHE[key]


def kernel(x, Wq, bq, Wk, bk, Wv, bv, W1, b1, W2, b2, g1, be1, g2, be2):
    """Full-input entry: x [8, 1024, 1024]; returns [8, 1024, 1024] float32.

    Shards batch across the 8 NeuronCores (one element per core), runs the
    Bass decoder kernel SPMD, and gathers/transposes the per-core outputs.
    """
    from concourse.bass_utils import run_bass_kernel_spmd

    x = np.asarray(x, np.float32)
    B = x.shape[0]
    bv_e = np.asarray(be1, np.float32) @ np.asarray(Wv, np.float32) + \
        np.asarray(bv, np.float32)
    nc = _get_nc(bias_free=bool(np.all(bv_e == 0.0)))
    args = tuple(np.asarray(a, np.float32) for a in
                 (Wq, bq, Wk, bk, Wv, bv, W1, b1, W2, b2, g1, be1, g2, be2))
    in_maps = [host_inputs(x[c], *args) for c in range(B)]
    res = run_bass_kernel_spmd(nc, in_maps, core_ids=list(range(B)))
    out = np.stack([np.asarray(r["yT"], np.float32).T for r in res.results],
                   axis=0)
    return np.ascontiguousarray(out, np.float32)
